# revision 57
# baseline (speedup 1.0000x reference)
"""Trainium2 Bass kernel for CustomPositionsPiecewiseConv2d.

Math: for knots positions=[-1,-.5,0,.5,1] and x in [0,1], the active
interpolation coefficients are c2 = relu(1-2v), c4 = relu(2v-1),
c3 = 1 - c2 - c4 (exactly, everywhere incl. the zero-padding border), so
    out = C2 (x) (W2-W3) + C4 (x) (W4-W3) + sum_ck W3[o,c,k] + bias
Each plane is elementwise in v; the 3x3 im2col becomes shifted access-pattern
reads feeding PSUM-accumulated matmuls.  bf16 rounding absorbs the
isclose(v,1) mask (relu(2v-1) rounds to exactly 1.0 there); total rel err
~1e-3 vs the 2e-2 gate.

Layouts:
  x arrives pre-padded bf16 (phi computes plane borders from x=0: c2=1,
  c4=0 -- no memsets on the gather path).  PLB [2C, 2, HP, WP] bf16 keeps
  the plane index as a FREE dim, so one DMA gathers the per-image,
  channel-interleaved Y lo half = [c2/c4 x 32ch]; weights are
  row-interleaved on host to match (row 2c+g = plane-g, chan c).  The Y hi
  half is the planes shifted one column via a flat-view copy (contiguous
  runs; the row-wrap garbage lands in a never-read column).  Taps (r,0) and
  (r,1) pair into K=128 matmuls; taps (r,2) read the hi half at +1 col with
  a zero lo-half lhs -- every matmul is K=128, which the HAM clock governor
  requires for full rate (K=64 streams are held at half clock; fp32r is
  power-capped to half clock outright).

Pipeline: phi/gather/GEMM/drain/store are banded over image rows and
software-pipelined; dummy K=128 warmup matmuls bridge the fill so HAM never
sees an idle PE.  DMA issue (~0.6us each), completion receipts (~1-2us),
and the 8 completion-sem lanes per engine (lane reuse couples unrelated
DMAs) bound the fill, so transfers are spread over the sync, scalar, and
gpsimd rings: sync = x + weights + early gathers + half the outs, gpsimd =
slack-tolerant gathers, scalar = the other outs.

Sharding: data-parallel over batch, 2 images per core on 8 cores.
"""

import numpy as np

B, C, H, W = 16, 32, 64, 64
O, P, KH, KW = 128, 5, 3, 3
NCORES = 8
IPC = B // NCORES            # images per core
HP, WP = H + 2, W + 2        # padded image (pad=1)
RT = 8                       # output rows per PSUM tile
NT = H // RT                 # PSUM tiles per image
GR = 2                       # tiles per drain group
NG = NT // GR                # groups per image
K2 = KH * KW
ATOL = 1e-5
RTOL = 1e-5

PAIR = True                  # pair taps (r,0)+(r,1) into K=128 matmuls
WARM = 24                    # PE warmup matmuls (clock ramp)

# phi/gather row chunks (padded-row bands); band 0 is small so the first
# gather (and matmul) starts early
BANDS = [(0, 10), (10, 18), (18, 34), (34, 50), (50, 66)]
GROUPS = [(t, t + 1) for t in range(NT)]  # PSUM-tile ranges per drain group


# ---------------------------------------------------------------- host math


def _isclose_np(a, b):
    return np.abs(a - b) <= np.float32(ATOL) + np.float32(RTOL) * np.abs(b)


def _reference_np(x, weights, bias, positions):
    """Direct numpy port of the reference (fallback path)."""
    EPS = 1e-6
    Bn, Cn, Hn, Wn = x.shape
    On, _, Pn, KHn, KWn = weights.shape
    xp = np.pad(x, ((0, 0), (0, 0), (1, 1), (1, 1)))
    cols = [
        xp[:, :, i : i + Hn, j : j + Wn] for i in range(KHn) for j in range(KWn)
    ]
    pat = np.stack(cols, axis=2)
    v = pat.reshape(Bn, Cn, KHn * KWn, Hn * Wn).astype(np.float32)

    left, right = positions[:-1], positions[1:]
    denom = right - left
    denom = np.where(denom == 0, np.float32(EPS), denom)
    varc = (1.0 / denom).astype(np.float32)
    const = (-left * varc).astype(np.float32)

    m_first = _isclose_np(v, positions[0])
    m_last = _isclose_np(v, positions[-1])
    in_range = (~(m_first | m_last)) & (v >= positions[0]) & (v <= positions[-1])

    coeff = np.zeros(v.shape + (Pn,), np.float32)
    coeff[..., 0] += m_first.astype(np.float32)
    coeff[..., Pn - 1] += m_last.astype(np.float32)
    for p in range(Pn - 1):
        m = (in_range & (v >= positions[p]) & (v < positions[p + 1])).astype(
            np.float32
        )
        t = v * varc[p] + const[p]
        coeff[..., p] += m * (1.0 - t)
        coeff[..., p + 1] += m * t

    Wk = np.transpose(weights, (0, 1, 3, 4, 2)).reshape(On, Cn, KHn * KWn, Pn)
    ident = np.all(np.abs(Wk - 1.0) <= np.float32(ATOL + RTOL), axis=-1)
    Wk_eff = np.where(ident[..., None], np.float32(0.0), Wk)

    out = np.einsum("bcklp,ockp->bol", coeff, Wk_eff, optimize=True)
    out = out + np.einsum(
        "bckl,ock->bol", v, ident.astype(np.float32), optimize=True
    )
    out = out + bias[None, :, None]
    return out.reshape(Bn, On, Hn, Wn).astype(np.float32)


def _host_weights(weights, bias):
    """Fold c3 away and interleave rows to match the device plane layout.

    Returns (winter [2C, K2, O] f32 with row 2c+g = (W{2,4}-W3)[:,c,k].T,
    bias_eff [O] f32 = bias + sum_ck W3, ident_any)."""
    Wk = np.transpose(weights, (0, 1, 3, 4, 2)).reshape(O, C, K2, P)
    ident = np.all(np.abs(Wk - 1.0) <= np.float32(ATOL + RTOL), axis=-1)
    ident_any = bool(ident.any())
    Wk_eff = np.where(ident[..., None], np.float32(0.0), Wk)
    W3 = Wk_eff[:, :, :, 3].astype(np.float64)
    W2 = Wk_eff[:, :, :, 2].astype(np.float64) - W3   # c2 weights [O,C,K2]
    W4 = Wk_eff[:, :, :, 4].astype(np.float64) - W3   # c4 weights
    winter = np.zeros((2 * C, K2, O), np.float32)
    winter[0::2] = W2.astype(np.float32).transpose(1, 2, 0)
    winter[1::2] = W4.astype(np.float32).transpose(1, 2, 0)
    bias_eff = (bias.astype(np.float64) + W3.sum(axis=(1, 2))).astype(np.float32)
    return winter, np.ascontiguousarray(bias_eff), ident_any


def _pack_weights(winter):
    """Device weight tensors (bf16) for the chosen tap schedule."""
    import ml_dtypes

    bf = ml_dtypes.bfloat16
    if not PAIR:
        return {"wint": np.ascontiguousarray(winter.astype(bf))}
    # pair pass r: lo rows = tap (r,0), hi rows = tap (r,1);
    # single pass r: tap (r,2) read from the hi (shifted) Y half
    # singles are padded to K=128 with a zero lo half: full PE row
    # utilization keeps the HAM clock governor at k=8 (K=64 streams are
    # held at half clock)
    wpair = np.zeros((4 * C, KH, O), np.float32)
    wsing = np.zeros((4 * C, KH, O), np.float32)
    for r in range(KH):
        wpair[0 : 2 * C, r] = winter[:, r * KW + 0]
        wpair[2 * C : 4 * C, r] = winter[:, r * KW + 1]
        wsing[2 * C : 4 * C, r] = winter[:, r * KW + 2]
    return {
        "wpair": np.ascontiguousarray(wpair.astype(bf)),
        "wsing": np.ascontiguousarray(wsing.astype(bf)),
    }


# ---------------------------------------------------------------- device IR


def _build_nc():
    import concourse.tile as tile
    from concourse import bacc, mybir

    f32 = mybir.dt.float32
    bf16 = mybir.dt.bfloat16
    Alu = mybir.AluOpType
    Act = mybir.ActivationFunctionType

    nc = bacc.Bacc("TRN2", target_bir_lowering=False, debug=False,
                   num_devices=NCORES)
    # x arrives pre-padded (zeros border) so phi computes the plane borders
    # from x=0 directly: c2=relu(1-0)=1, c4=relu(0-1)=0 -- no border memsets
    x_d = nc.dram_tensor("x", [IPC, C, HP, WP], bf16, kind="ExternalInput").ap()
    if PAIR:
        wp_d = nc.dram_tensor("wpair", [4 * C, KH, O], bf16,
                              kind="ExternalInput").ap()
        ws_d = nc.dram_tensor("wsing", [4 * C, KH, O], bf16,
                              kind="ExternalInput").ap()
    else:
        wi_d = nc.dram_tensor("wint", [2 * C, K2, O], bf16,
                              kind="ExternalInput").ap()
    b_d = nc.dram_tensor("bias", [O, 1], f32, kind="ExternalInput").ap()
    o_d = nc.dram_tensor("out", [IPC, O, H, W], bf16,
                         kind="ExternalOutput").ap()

    YPART = 4 * C if PAIR else 2 * C

    with tile.TileContext(nc) as tc:
        with (
            tc.tile_pool(name="const", bufs=1) as constp,
            tc.tile_pool(name="scratch", bufs=1) as scrp,
            tc.tile_pool(name="ybuf", bufs=2) as ybufp,
            tc.tile_pool(name="psum", bufs=1, space="PSUM") as psump,
            tc.tile_pool(name="osb", bufs=4) as osbp,
        ):
            XF = scrp.tile([IPC * C, HP, WP], bf16)
            # x row-band 0 for both images first (phi critical path)
            nc.sync.dma_start(XF[:, 0:18], x_d[:, :, 0:18])

            # ACT table preload + phi operands first: phi must not wait on
            # the border memsets below (gpsimd runs in issue order)
            tiny = constp.tile([IPC * C, 1], f32)
            nc.gpsimd.memset(tiny[:], 0.0)
            nc.scalar.activation(tiny[:], tiny[:], Act.Relu, bias=0.0, scale=1.0)
            negone = constp.tile([IPC * C, 1], f32)
            nc.gpsimd.memset(negone[:], -1.0)

            # PE clock ramp: dummy matmuls (results never read); use the last
            # group's PSUM banks so group 0 isn't blocked on the warm drain.
            zb = constp.tile([128, 512], bf16)
            nc.gpsimd.memset(zb[:], 0.0)
            pw = [psump.tile([O, 512], f32, name=f"ps_warm{k}",
                             tag=f"ps{6 + k}") for k in range(2)]
            for j in range(WARM):
                nc.tensor.matmul(pw[j % 2][:], zb[0:128, 0:128], zb[:],
                                 start=(j < 2), stop=(j >= WARM - 2))

            # weights + bias + rest of x
            if PAIR:
                wp_sb = constp.tile([4 * C, KH, O], bf16)
                nc.sync.dma_start(wp_sb[:], wp_d[:])
                ws_sb = constp.tile([4 * C, KH, O], bf16)
                nc.sync.dma_start(ws_sb[:], ws_d[:])
            else:
                wi_sb = constp.tile([2 * C, K2, O], bf16)
                nc.sync.dma_start(wi_sb[:], wi_d[:])
            b_sb = constp.tile([O, 1], f32)
            nc.sync.dma_start(b_sb[:], b_d[:])
            nc.sync.dma_start(XF[:, 18:HP], x_d[:, :, 18:HP])

            # coefficient planes, plane index as free dim: [2C, {c2,c4}, HP, WP]
            PLB = scrp.tile([IPC * C, 2, HP, WP], bf16)

            def phi_band(pr0, pr1):
                """c2/c4 planes for padded rows [pr0,pr1) (both images)."""
                xf = XF[:, pr0:pr1]
                nc.scalar.activation(PLB[:, 1, pr0:pr1, :], xf,
                                     Act.Relu, bias=negone[:], scale=2.0)
                nc.scalar.activation(PLB[:, 0, pr0:pr1, :], xf,
                                     Act.Relu, bias=1.0, scale=-2.0)

            def gather(Y, i, pr0, pr1, eng):
                """One DMA: Y[0:2C, rows] = channel-interleaved c2/c4 of
                image i; PAIR adds the col+1-shifted copy in the hi half."""
                src = PLB[i * C : (i + 1) * C, :, pr0:pr1]
                eng.dma_start(Y[0 : 2 * C, pr0:pr1], src)
                if PAIR:
                    # hi half = planes shifted one col: flat views keep the
                    # runs contiguous (64 descriptors, not 64*rows); the one
                    # row-wrap garbage element lands in col WP-1, never read
                    f0, f1 = pr0 * WP, pr1 * WP
                    dst = Y[2 * C : 4 * C].rearrange("p h w -> p (h w)")
                    hsrc = PLB[i * C : (i + 1) * C].rearrange(
                        "p g h w -> p g (h w)"
                    )
                    eng.dma_start(dst[:, f0 : f1 - 1],
                                  hsrc[:, :, f0 + 1 : f1])

            def mm_tile(Y, ps, t, first, last):
                """All tap passes for PSUM tile t (output rows RT*t..+RT)."""
                if PAIR:
                    # pair pass r: taps (r,0)+(r,1); single pass r: tap (r,2)
                    # via the shifted hi half read at +1 col (lo weights are
                    # zero) -- every matmul is K=128
                    for r in range(KH):
                        rows = slice(t * RT + r, t * RT + r + RT)
                        nc.tensor.matmul(ps[:], wp_sb[:, r, :],
                                         Y[:, rows, 0:W],
                                         start=(first and r == 0), stop=False)
                    for r in range(KH):
                        rows = slice(t * RT + r, t * RT + r + RT)
                        nc.tensor.matmul(ps[:], ws_sb[:, r, :],
                                         Y[:, rows, 1 : W + 1],
                                         start=False,
                                         stop=(last and r == KH - 1))
                else:
                    for ki in range(K2):
                        kh, kw = divmod(ki, KW)
                        rows = slice(t * RT + kh, t * RT + kh + RT)
                        nc.tensor.matmul(ps[:], wi_sb[:, ki, :],
                                         Y[:, rows, kw : kw + W],
                                         start=(first and ki == 0),
                                         stop=(last and ki == K2 - 1))

            # image 0's gathers ride sync, interleaved with the bulk-x
            # chunks that unblock the next phi band; image 1's (deadline
            # ~10us later) ride the gpsimd SWDGE ring so sync is free for
            # the output stores
            Ys = [
                ybufp.tile([YPART, HP, WP], bf16, name="Y", tag=f"Y{i}")
                for i in range(IPC)
            ]
            # sync keeps only bands 0/2 of image 0: the sync ring has just 8
            # completion-sem lanes, and lane reuse transitively couples
            # unrelated DMAs (each edge costs the ~1-2us receipt latency).
            # The rest ride gpsimd, whose deadlines are loose.
            for j, (pr0, pr1) in enumerate(BANDS):
                phi_band(pr0, pr1)
                gather(Ys[0], 0, pr0, pr1,
                       nc.sync if j in (0, 2) else nc.gpsimd)
            for pr0, pr1 in BANDS:
                gather(Ys[1], 1, pr0, pr1, nc.gpsimd)
            for i in range(IPC):
                Y = Ys[i]
                for gi, (t0, t1) in enumerate(GROUPS):
                    osb = osbp.tile([O, t1 - t0, RT * W], bf16, name="osb")
                    for t in range(t0, t1):
                        ps = psump.tile([O, RT * W], f32, name=f"ps{t}",
                                        tag=f"ps{t}")
                        mm_tile(Y, ps, t, first=True, last=True)
                        # alternate drain engines so both tiles of a group
                        # drain concurrently (scalar is free once phi ends)
                        if t % 2 == 0:
                            nc.vector.tensor_scalar(osb[:, t - t0], ps[:],
                                                    b_sb[:, 0:1], None,
                                                    Alu.add)
                        else:
                            nc.scalar.activation(osb[:, t - t0], ps[:],
                                                 Act.Identity,
                                                 bias=b_sb[:, 0:1], scale=1.0)
                    # alternate rings so two output stores are in flight
                    oeng = nc.sync if gi % 2 == 0 else nc.scalar
                    oeng.dma_start(
                        o_d[i, :, t0 * RT : t1 * RT, :],
                        osb[:].rearrange("o g (r w) -> o (g r) w", r=RT),
                    )
    nc.compile()
    return nc


# ---------------------------------------------------------------- entry


def _prep(inputs):
    x = np.ascontiguousarray(np.asarray(inputs["x"], dtype=np.float32))
    weights = np.ascontiguousarray(np.asarray(inputs["weights"], dtype=np.float32))
    bias = np.ascontiguousarray(np.asarray(inputs["bias"], dtype=np.float32))
    positions = np.ascontiguousarray(
        np.asarray(inputs["positions"], dtype=np.float32)
    )
    return x, weights, bias, positions


def _fast_path_ok(x, positions):
    expect = np.linspace(-1.0, 1.0, P, dtype=np.float32)
    return (
        x.shape == (B, C, H, W)
        and positions.shape == (P,)
        and np.array_equal(positions, expect)
        and float(x.min()) >= 0.0
        and float(x.max()) <= 1.0
    )


def kernel(**inputs):
    x, weights, bias, positions = _prep(inputs)
    if not _fast_path_ok(x, positions):
        return _reference_np(x, weights, bias, positions)

    winter, bias_eff, ident_any = _host_weights(weights, bias)
    if ident_any:
        # identity-shortcut weights present: needs the raw-v plane; use the
        # exact fallback rather than a rarely-exercised device path
        return _reference_np(x, weights, bias, positions)

    from concourse.bass_utils import run_bass_kernel_spmd

    import ml_dtypes

    nc = _build_nc()
    wmap = _pack_weights(winter)
    bias2d = np.ascontiguousarray(bias_eff.reshape(O, 1))
    xp = np.pad(x, ((0, 0), (0, 0), (1, 1), (1, 1)))
    xbf = np.ascontiguousarray(xp.astype(ml_dtypes.bfloat16))
    in_maps = [
        {"x": xbf[i * IPC : (i + 1) * IPC],
         "bias": bias2d, **wmap}
        for i in range(NCORES)
    ]
    res = run_bass_kernel_spmd(nc, in_maps, core_ids=list(range(NCORES)))
    out = np.concatenate([res.results[i]["out"] for i in range(NCORES)], axis=0)
    return np.ascontiguousarray(out.astype(np.float32))


# ------------------------------------------------------------ dev utilities


def _run_sim(inputs):
    """CoreSim single-core run (images 0..IPC-1) for correctness debugging."""
    from concourse.bass_interp import CoreSim

    x, weights, bias, positions = _prep(inputs)
    assert _fast_path_ok(x, positions)
    winter, bias_eff, ident_any = _host_weights(weights, bias)
    assert not ident_any
    import ml_dtypes

    nc = _build_nc()
    sim = CoreSim(nc)
    xp = np.pad(x[:IPC], ((0, 0), (0, 0), (1, 1), (1, 1)))
    sim.tensor("x")[:] = xp.astype(ml_dtypes.bfloat16)
    for k, v in _pack_weights(winter).items():
        sim.tensor(k)[:] = v
    sim.tensor("bias")[:] = bias_eff.reshape(O, 1)
    sim.simulate()
    return np.array(sim.tensor("out"))
